# revision 8
# baseline (speedup 1.0000x reference)
"""Cross-attention kernel for Trainium2 (Bass/Tile), 8 NeuronCores — v4.

Transpose-free formulation: mm1 computes S^T = ref @ dom^T directly
(lhsT = refT chunk, rhs = domT chunk), so the exp output is already P^T
in the [key, query] orientation mm2 needs as lhsT — no PE transposes,
no PSUM->SBUF copies of P, no scalar accumulator reads.

Softmax row sums are fused into mm2: the moving operand is ref augmented
with a ones column, split A/B to fit PSUM banks (A = ref[:, :256] + ones
-> [128,257], B = ref[:, 256:] -> [128,256]). Column 256 of the A tile is
the per-query rowsum, already in per-partition layout: reciprocal + two
tensor_scalar_muls normalize x during eviction. bf16 moving/stationary
operands make the short streams viable (1 cyc/row at any free size, and
the ~53ns bf16 weight load hides under a 107ns stream; fp32r would be
weight-port-bound below ~450 rows).

Per batch b:
    S^T = (ref @ dom^T) * SCALE     [N, N]   16 psum tiles [128,512], fp32r
    P^T = exp(S^T)                  bf16, straight from the scalar engine
    x   = P @ ref_aug               bf16 matmuls; rowsum rides in col 256
    out[2*cp+e, j] = sum_q x[512e+q, cp] wt[q, j] + bias[j]
        (scramble+linear fused: x tiles are mm3's lhsT in natural layout)

All matmul operands are bf16 (hosts cast inputs, halving DMA bytes; the
QK^T logit error is a ~0.06-absolute random walk over the 512-term
contraction and softmax normalization absorbs the common mode — measured
end-to-end rel err 5.9e-3 vs the 2e-2 gate). PSUM accumulation and the
output stay fp32.

Sharding: data-parallel over batch. B=16 -> 2 batches per core, no
collectives. DMA: Q0 (gpsimd SWDGE) streams domT/wt/bias, Q1 (sync HWDGE)
refT/refA/refB; out stores alternate across both queues. Batch-0 loads
are k-chunk granular so the first mm1 accumulation starts on the earliest
512KB. PSUM: mm1+mm3 share a 3-bank pool, mm2 A/B use 2+2.
"""

import os
from contextlib import ExitStack

import ml_dtypes
import numpy as np

import concourse.bass as bass
import concourse.mybir as mybir
import concourse.tile as tile
from concourse import bacc
from concourse._compat import with_exitstack
from concourse.bass_utils import run_bass_kernel_spmd

B, N, C = 16, 1024, 512
NUM_HEADS = 8
SCALE = (C // NUM_HEADS) ** -0.5  # 0.125
CORES = 8
BPC = B // CORES  # batches per core

P = 128          # partitions
CCH = C // P     # 4 contraction chunks over channels
MH = N // 512    # 2 query halves
MCH = N // P     # 8 key chunks
JT = C // P      # 4 output-column blocks per half
CA = C // 2 + 1  # 257: A-tile free size (256 ref channels + ones)
CB = C // 2      # 256: B-tile free size

F32 = mybir.dt.float32
F32R = mybir.dt.float32r
BF16 = mybir.dt.bfloat16

USE_F32R = os.environ.get("KERNEL_F32R", "1") == "1"
WARMUP_MMS = int(os.environ.get("KERNEL_WARMUP", "22"))


def _r(ap):
    return ap.bitcast(F32R) if USE_F32R else ap


@with_exitstack
def _core_kernel(ctx: ExitStack, tc: tile.TileContext,
                 domt_d, reft_d, refa_d, refb_d, wt_d, bias_d,
                 biasbf_d, out_d):
    nc = tc.nc

    consts = ctx.enter_context(tc.tile_pool(name="consts", bufs=1))

    ps_S = ctx.enter_context(tc.tile_pool(name="ps_s", bufs=3, space="PSUM"))
    ps_A = ctx.enter_context(tc.tile_pool(name="ps_a", bufs=2, space="PSUM"))
    ps_B = ctx.enter_context(tc.tile_pool(name="ps_b", bufs=2, space="PSUM"))

    # PE warmup: dependency-free matmuls on memset zeros while the first
    # input DMAs stream, so the HAM clock gate reaches full rate before
    # real work arrives. The source comes from a gpsimd memset (runs at
    # ~6us) — a vector-engine producer would gate the warmup behind the
    # DVE table load and start it ~2.5us later.
    if WARMUP_MMS:
        zw = consts.tile([P, 640], BF16)
        nc.gpsimd.memset(zw[:], 0.0)
        warm_ps = ps_S.tile([P, 512], F32, tag="ps_s")
        for i in range(WARMUP_MMS):
            nc.tensor.matmul(warm_ps[:], zw[:, :P], zw[:, P:640],
                             start=True, stop=True)

    p_refA = ctx.enter_context(tc.tile_pool(name="refA", bufs=2))
    p_refB = ctx.enter_context(tc.tile_pool(name="refB", bufs=2))
    p_domT = ctx.enter_context(tc.tile_pool(name="domT", bufs=2))
    p_refT = ctx.enter_context(tc.tile_pool(name="refT", bufs=2))
    p_Pt = ctx.enter_context(tc.tile_pool(name="probsT", bufs=3))
    p_x = ctx.enter_context(tc.tile_pool(name="x", bufs=8))
    p_out = ctx.enter_context(tc.tile_pool(name="out", bufs=4))
    p_stats = ctx.enter_context(tc.tile_pool(name="stats", bufs=8))

    # ---- pre-emit every input DMA so the rings stream continuously ----
    def load_T(sb, dr, b, eng, k_granular=False):
        # [C, N] host-pretransposed tensor: chunk k lands at cols
        # [k*N, (k+1)*N); within a chunk, key/query index is the column.
        for h in range(MH):
            if k_granular:
                for k in range(CCH):
                    eng.dma_start(
                        sb[:, k * N + h * 512: k * N + (h + 1) * 512],
                        dr[b, k * P:(k + 1) * P, h * 512:(h + 1) * 512],
                    )
            else:
                eng.dma_start(
                    sb[:, :].rearrange("p (k n) -> p k n", k=CCH)
                    [:, :, h * 512:(h + 1) * 512],
                    dr[b, :, h * 512:(h + 1) * 512]
                    .rearrange("(k p) c -> p k c", p=P),
                )

    domT_sbs = [p_domT.tile([P, CCH * N], BF16, tag="domT", name=f"domT_sb{i}")
                for i in range(BPC)]
    refT_sbs = [p_refT.tile([P, CCH * N], BF16, tag="refT", name=f"refT_sb{i}")
                for i in range(BPC)]
    refA_sbs = [p_refA.tile([P, MCH * CA], BF16, tag="refA",
                            name=f"refA_sb{i}") for i in range(BPC)]
    refB_sbs = [p_refB.tile([P, MCH * CB], BF16, tag="refB",
                            name=f"refB_sb{i}") for i in range(BPC)]

    def load_ref(b, eng):
        # key-chunk mi of ref[:, :256] -> refA block mi cols 0..255 (col 256
        # holds the memset ones), ref[:, 256:] -> refB block mi
        nc.vector.memset(
            refA_sbs[b][:, :].rearrange("p (t c) -> p t c", t=MCH)[:, :, CB:],
            1.0)
        eng.dma_start(
            refA_sbs[b][:, :].rearrange("p (t c) -> p t c", t=MCH)[:, :, :CB],
            refa_d[b].rearrange("(t p) c -> p t c", p=P),
        )
        eng.dma_start(
            refB_sbs[b][:, :].rearrange("p (t c) -> p t c", t=MCH),
            refb_d[b].rearrange("(t p) c -> p t c", p=P),
        )

    # Q1 (sync): refT0, refA0/refB0, refT1, refA1/refB1
    load_T(refT_sbs[0], reft_d, 0, nc.sync, k_granular=True)
    load_ref(0, nc.sync)
    # Q0 (gpsimd): domT0, wt, domT1, bias
    load_T(domT_sbs[0], domt_d, 0, nc.gpsimd, k_granular=True)
    wt_sb = consts.tile([P, CCH * C], BF16)
    nc.gpsimd.dma_start(
        wt_sb[:, :].rearrange("p (q c) -> p q c", q=CCH),
        wt_d.rearrange("(q p) c -> p q c", p=P),
    )
    if BPC > 1:
        load_T(refT_sbs[1], reft_d, 1, nc.sync)
        load_ref(1, nc.sync)
        load_T(domT_sbs[1], domt_d, 1, nc.gpsimd)
    bias_sb = consts.tile([P, C], F32)
    nc.gpsimd.dma_start(bias_sb[:], bias_d.partition_broadcast(P))
    # rank-1 bias injection operands: ps_z starts at ones^T @ brow = bias
    # broadcast, so the tail groups evict with a scalar copy instead of a
    # vector add (vector is busy with x evictions at the end)
    ones_row = consts.tile([1, P], BF16)
    nc.gpsimd.memset(ones_row[:], 1.0)
    brow_sb = consts.tile([1, C], BF16)
    nc.gpsimd.dma_start(brow_sb[:], biasbf_d[None, :])

    for b in range(BPC):
        domT_sb = domT_sbs[b]
        refT_sb = refT_sbs[b]
        refA_sb = refA_sbs[b]
        refB_sb = refB_sbs[b]

        out_v = out_d[b].rearrange("(n2 two) j -> two n2 j", two=2)

        Pt_sbs = {}
        x_tiles = []

        def mm1_group(h, mi):
            # S^T tile [key(mi) 128, query 512] -> exp into bf16 Pt
            if mi == 0:
                Pt_sbs[h] = p_Pt.tile([P, MCH * 512], BF16, tag="probsT",
                                      name=f"Pt_sb{b}_{h}")
            Pt_sb = Pt_sbs[h]
            ps_s = ps_S.tile([P, 512], F32, tag="ps_s",
                             name=f"ps_s{b}_{h}_{mi}")
            for k in range(CCH):
                nc.tensor.matmul(
                    ps_s[:],
                    refT_sb[:, k * N + mi * P: k * N + (mi + 1) * P],
                    domT_sb[:, k * N + h * 512: k * N + (h + 1) * 512],
                    start=(k == 0), stop=(k == CCH - 1),
                )
            nc.scalar.activation(Pt_sb[:, mi * 512:(mi + 1) * 512],
                                 ps_s[:],
                                 mybir.ActivationFunctionType.Exp,
                                 scale=float(SCALE))

        def mm2_group(h, nl):
            # x tile [query 128, C] = sum_mi Pt(mi)^T @ ref_aug chunk;
            # rowsum rides in column 256 of the A accumulator
            Pt_sb = Pt_sbs[h]
            ps_a = ps_A.tile([P, CA], F32, tag="ps_a",
                             name=f"ps_a{b}_{h}_{nl}")
            ps_b = ps_B.tile([P, CB], F32, tag="ps_b",
                             name=f"ps_b{b}_{h}_{nl}")
            for mi in range(MCH):
                lhsT = Pt_sb[:, mi * 512 + nl * P: mi * 512 + (nl + 1) * P]
                nc.tensor.matmul(ps_a[:], lhsT,
                                 refA_sb[:, mi * CA:(mi + 1) * CA],
                                 start=(mi == 0), stop=(mi == MCH - 1))
                nc.tensor.matmul(ps_b[:], lhsT,
                                 refB_sb[:, mi * CB:(mi + 1) * CB],
                                 start=(mi == 0), stop=(mi == MCH - 1))
            # normalize on eviction: recip of col 256, scale both halves
            recip = p_stats.tile([P, 1], F32, tag="recip",
                                 name=f"recip{b}_{h}_{nl}")
            nc.vector.reciprocal(recip[:], ps_a[:, CB:CA])
            x_t = p_x.tile([P, C], BF16, tag="x", name=f"x_t{b}_{h}_{nl}")
            nc.vector.tensor_scalar_mul(x_t[:, :CB], ps_a[:, :CB], recip[:])
            nc.vector.tensor_scalar_mul(x_t[:, CB:], ps_b[:], recip[:])
            x_tiles.append(x_t)

        def mm3_group(e, cb):
            # out rows (2*cp + e) = x_half_e^T @ wt + bias
            pe_bias = b == BPC - 1 and e == 1 and cb >= JT - 2
            if b == BPC - 1 and e == 1 and cb == JT - 1:
                # final group: accumulate in two 256-col PSUM halves (the
                # mm2 A/B banks are free by now) so the first half's store
                # launches while the second half is still accumulating,
                # and the last copy->store chain is half as long
                o_sb = p_out.tile([P, C], F32, tag="out",
                                  name=f"o_sb{b}_{e}_{cb}")
                rows = out_v[e, cb * P:(cb + 1) * P]
                for half, (pool, eng) in enumerate(
                        ((ps_A, nc.gpsimd), (ps_B, nc.sync))):
                    sl = slice(half * CB, (half + 1) * CB)
                    ps_h = pool.tile([P, CA if half == 0 else CB], F32,
                                     tag="ps_a" if half == 0 else "ps_b",
                                     name=f"ps_zf{half}")
                    nc.tensor.matmul(ps_h[:, :CB], ones_row[:],
                                     brow_sb[:, sl], start=True, stop=False)
                    for q in range(CCH):
                        x_t = x_tiles[e * CCH + q]
                        nc.tensor.matmul(
                            ps_h[:, :CB],
                            x_t[:, cb * P:(cb + 1) * P],
                            wt_sb[:, q * C + half * CB: q * C + half * CB
                                  + CB],
                            start=False, stop=(q == CCH - 1),
                        )
                    nc.scalar.copy(o_sb[:, sl], ps_h[:, :CB])
                    eng.dma_start(rows[:, sl], o_sb[:, sl])
                return
            ps_z = ps_S.tile([P, C], F32, tag="ps_s",
                             name=f"ps_z{b}_{e}_{cb}")
            if pe_bias:
                nc.tensor.matmul(ps_z[:], ones_row[:], brow_sb[:],
                                 start=True, stop=False)
            for q in range(CCH):
                x_t = x_tiles[e * CCH + q]
                nc.tensor.matmul(
                    ps_z[:],
                    x_t[:, cb * P:(cb + 1) * P],
                    wt_sb[:, q * C:(q + 1) * C],
                    start=False if pe_bias else (q == 0),
                    stop=(q == CCH - 1),
                )
            o_sb = p_out.tile([P, C], F32, tag="out",
                              name=f"o_sb{b}_{e}_{cb}")
            if pe_bias:
                nc.scalar.copy(o_sb[:], ps_z[:])
                eng = nc.gpsimd if (e * JT + cb) % 2 == 0 else nc.sync
                eng.dma_start(out_v[e, cb * P:(cb + 1) * P, :], o_sb[:])
            else:
                nc.vector.tensor_add(o_sb[:], ps_z[:], bias_sb[:])
                eng = nc.gpsimd if (e * JT + cb) % 2 == 0 else nc.sync
                eng.dma_start(out_v[e, cb * P:(cb + 1) * P, :], o_sb[:])

        for mi in range(MCH):
            mm1_group(0, mi)
        for mi in range(MCH):
            mm1_group(1, mi)
        for nl in range(4):
            mm2_group(0, nl)
        for cb in range(JT):
            mm3_group(0, cb)
        for nl in range(4):
            mm2_group(1, nl)
        for cb in range(JT):
            mm3_group(1, cb)


_CACHED = {}


def _build():
    key = ("nc", USE_F32R, WARMUP_MMS)
    if key in _CACHED:
        return _CACHED[key]
    nc = bacc.Bacc("TRN2", target_bir_lowering=False, debug=False)
    domt_d = nc.dram_tensor("domt", [BPC, C, N], BF16, kind="ExternalInput").ap()
    reft_d = nc.dram_tensor("reft", [BPC, C, N], BF16, kind="ExternalInput").ap()
    refa_d = nc.dram_tensor("refa", [BPC, N, CB], BF16,
                            kind="ExternalInput").ap()
    refb_d = nc.dram_tensor("refb", [BPC, N, CB], BF16,
                            kind="ExternalInput").ap()
    wt_d = nc.dram_tensor("wt", [C, C], BF16, kind="ExternalInput").ap()
    bias_d = nc.dram_tensor("bias", [C], F32, kind="ExternalInput").ap()
    biasbf_d = nc.dram_tensor("biasbf", [C], BF16, kind="ExternalInput").ap()
    out_d = nc.dram_tensor("out", [BPC, N, C], F32, kind="ExternalOutput").ap()

    with tile.TileContext(nc) as tc:
        _core_kernel(tc, domt_d, reft_d, refa_d, refb_d, wt_d, bias_d,
                     biasbf_d, out_d)
    nc.compile()
    _CACHED[key] = nc
    return nc


LAST_RESULTS = None


def kernel(dom, ref, proj_w, proj_b):
    global LAST_RESULTS
    dom = np.ascontiguousarray(np.asarray(dom, dtype=np.float32))
    ref = np.ascontiguousarray(np.asarray(ref, dtype=np.float32))
    wt = np.ascontiguousarray(
        np.asarray(proj_w, dtype=np.float32).T.astype(ml_dtypes.bfloat16))
    bias = np.ascontiguousarray(np.asarray(proj_b, dtype=np.float32))

    ref_bf = ref.astype(ml_dtypes.bfloat16)
    refa = np.ascontiguousarray(ref_bf[:, :, :CB])
    refb = np.ascontiguousarray(ref_bf[:, :, CB:])
    domt = np.ascontiguousarray(
        dom.transpose(0, 2, 1).astype(ml_dtypes.bfloat16))
    reft = np.ascontiguousarray(
        ref.transpose(0, 2, 1).astype(ml_dtypes.bfloat16))
    nc = _build()
    in_maps = [
        {
            "domt": domt[c * BPC:(c + 1) * BPC],
            "reft": reft[c * BPC:(c + 1) * BPC],
            "refa": refa[c * BPC:(c + 1) * BPC],
            "refb": refb[c * BPC:(c + 1) * BPC],
            "wt": wt,
            "bias": bias,
            "biasbf": bias.astype(ml_dtypes.bfloat16),
        }
        for c in range(CORES)
    ]
    res = run_bass_kernel_spmd(nc, in_maps, list(range(CORES)))
    LAST_RESULTS = res
    if res.exec_time_ns is not None:
        print(f"HW exec time: {res.exec_time_ns} ns")
    return np.concatenate([r["out"] for r in res.results], axis=0)


# revision 9
# speedup vs baseline: 1.0067x; 1.0067x over previous
"""Cross-attention kernel for Trainium2 (Bass/Tile), 8 NeuronCores — v4.

Transpose-free formulation: mm1 computes S^T = ref @ dom^T directly
(lhsT = refT chunk, rhs = domT chunk), so the exp output is already P^T
in the [key, query] orientation mm2 needs as lhsT — no PE transposes,
no PSUM->SBUF copies of P, no scalar accumulator reads.

Softmax row sums are fused into mm2: the moving operand is ref augmented
with a ones column, split A/B to fit PSUM banks (A = ref[:, :256] + ones
-> [128,257], B = ref[:, 256:] -> [128,256]). Column 256 of the A tile is
the per-query rowsum, already in per-partition layout: reciprocal + two
tensor_scalar_muls normalize x during eviction. bf16 moving/stationary
operands make the short streams viable (1 cyc/row at any free size, and
the ~53ns bf16 weight load hides under a 107ns stream; fp32r would be
weight-port-bound below ~450 rows).

Per batch b:
    S^T = (ref @ dom^T) * SCALE     [N, N]   16 psum tiles [128,512], fp32r
    P^T = exp(S^T)                  bf16, straight from the scalar engine
    x   = P @ ref_aug               bf16 matmuls; rowsum rides in col 256
    out[2*cp+e, j] = sum_q x[512e+q, cp] wt[q, j] + bias[j]
        (scramble+linear fused: x tiles are mm3's lhsT in natural layout)

All matmul operands are bf16 (hosts cast inputs, halving DMA bytes; the
QK^T logit error is a ~0.06-absolute random walk over the 512-term
contraction and softmax normalization absorbs the common mode — measured
end-to-end rel err 5.9e-3 vs the 2e-2 gate). PSUM accumulation and the
output stay fp32.

Sharding: data-parallel over batch. B=16 -> 2 batches per core, no
collectives. DMA: Q0 (gpsimd SWDGE) streams domT/wt/bias, Q1 (sync HWDGE)
refT/refA/refB; out stores alternate across both queues. Batch-0 loads
are k-chunk granular so the first mm1 accumulation starts on the earliest
512KB. PSUM: mm1+mm3 share a 3-bank pool, mm2 A/B use 2+2.
"""

import os
from contextlib import ExitStack

import ml_dtypes
import numpy as np

import concourse.bass as bass
import concourse.mybir as mybir
import concourse.tile as tile
from concourse import bacc
from concourse._compat import with_exitstack
from concourse.bass_utils import run_bass_kernel_spmd

B, N, C = 16, 1024, 512
NUM_HEADS = 8
SCALE = (C // NUM_HEADS) ** -0.5  # 0.125
CORES = 8
BPC = B // CORES  # batches per core

P = 128          # partitions
CCH = C // P     # 4 contraction chunks over channels
MH = N // 512    # 2 query halves
MCH = N // P     # 8 key chunks
JT = C // P      # 4 output-column blocks per half
CA = C // 2 + 1  # 257: A-tile free size (256 ref channels + ones)
CB = C // 2      # 256: B-tile free size

F32 = mybir.dt.float32
F32R = mybir.dt.float32r
BF16 = mybir.dt.bfloat16

USE_F32R = os.environ.get("KERNEL_F32R", "1") == "1"
WARMUP_MMS = int(os.environ.get("KERNEL_WARMUP", "22"))


def _r(ap):
    return ap.bitcast(F32R) if USE_F32R else ap


@with_exitstack
def _core_kernel(ctx: ExitStack, tc: tile.TileContext,
                 domt_d, reft_d, refa_d, refb_d, wt_d, bias_d,
                 biasbf_d, out_d):
    nc = tc.nc

    consts = ctx.enter_context(tc.tile_pool(name="consts", bufs=1))

    ps_S = ctx.enter_context(tc.tile_pool(name="ps_s", bufs=3, space="PSUM"))
    ps_A = ctx.enter_context(tc.tile_pool(name="ps_a", bufs=2, space="PSUM"))
    ps_B = ctx.enter_context(tc.tile_pool(name="ps_b", bufs=2, space="PSUM"))

    # PE warmup: dependency-free matmuls on memset zeros while the first
    # input DMAs stream, so the HAM clock gate reaches full rate before
    # real work arrives. The source comes from a gpsimd memset (runs at
    # ~6us) — a vector-engine producer would gate the warmup behind the
    # DVE table load and start it ~2.5us later.
    if WARMUP_MMS:
        zw = consts.tile([P, 640], BF16)
        nc.gpsimd.memset(zw[:], 0.0)
        warm_ps = ps_S.tile([P, 512], F32, tag="ps_s")
        for i in range(WARMUP_MMS):
            nc.tensor.matmul(warm_ps[:], zw[:, :P], zw[:, P:640],
                             start=True, stop=True)

    p_refA = ctx.enter_context(tc.tile_pool(name="refA", bufs=2))
    p_refB = ctx.enter_context(tc.tile_pool(name="refB", bufs=2))
    p_domT = ctx.enter_context(tc.tile_pool(name="domT", bufs=2))
    p_refT = ctx.enter_context(tc.tile_pool(name="refT", bufs=2))
    p_Pt = ctx.enter_context(tc.tile_pool(name="probsT", bufs=3))
    p_x = ctx.enter_context(tc.tile_pool(name="x", bufs=8))
    p_out = ctx.enter_context(tc.tile_pool(name="out", bufs=4))
    p_stats = ctx.enter_context(tc.tile_pool(name="stats", bufs=8))

    # ---- pre-emit every input DMA so the rings stream continuously ----
    def load_T(sb, dr, b, eng, k_granular=False):
        # [C, N] host-pretransposed tensor: chunk k lands at cols
        # [k*N, (k+1)*N); within a chunk, key/query index is the column.
        for h in range(MH):
            if k_granular:
                for k in range(CCH):
                    eng.dma_start(
                        sb[:, k * N + h * 512: k * N + (h + 1) * 512],
                        dr[b, k * P:(k + 1) * P, h * 512:(h + 1) * 512],
                    )
            else:
                eng.dma_start(
                    sb[:, :].rearrange("p (k n) -> p k n", k=CCH)
                    [:, :, h * 512:(h + 1) * 512],
                    dr[b, :, h * 512:(h + 1) * 512]
                    .rearrange("(k p) c -> p k c", p=P),
                )

    domT_sbs = [p_domT.tile([P, CCH * N], BF16, tag="domT", name=f"domT_sb{i}")
                for i in range(BPC)]
    refT_sbs = [p_refT.tile([P, CCH * N], BF16, tag="refT", name=f"refT_sb{i}")
                for i in range(BPC)]
    refA_sbs = [p_refA.tile([P, MCH * CA], BF16, tag="refA",
                            name=f"refA_sb{i}") for i in range(BPC)]
    refB_sbs = [p_refB.tile([P, MCH * CB], BF16, tag="refB",
                            name=f"refB_sb{i}") for i in range(BPC)]

    def load_ref(b, eng):
        # key-chunk mi of ref[:, :256] -> refA block mi cols 0..255 (col 256
        # holds the memset ones), ref[:, 256:] -> refB block mi
        nc.vector.memset(
            refA_sbs[b][:, :].rearrange("p (t c) -> p t c", t=MCH)[:, :, CB:],
            1.0)
        eng.dma_start(
            refA_sbs[b][:, :].rearrange("p (t c) -> p t c", t=MCH)[:, :, :CB],
            refa_d[b].rearrange("(t p) c -> p t c", p=P),
        )
        eng.dma_start(
            refB_sbs[b][:, :].rearrange("p (t c) -> p t c", t=MCH),
            refb_d[b].rearrange("(t p) c -> p t c", p=P),
        )

    # Q1 (sync): refT0, refA0/refB0, refT1, refA1/refB1
    load_T(refT_sbs[0], reft_d, 0, nc.sync, k_granular=True)
    load_ref(0, nc.sync)
    # Q0 (gpsimd): domT0, wt, domT1, bias
    load_T(domT_sbs[0], domt_d, 0, nc.gpsimd, k_granular=True)
    wt_sb = consts.tile([P, CCH * C], BF16)
    nc.gpsimd.dma_start(
        wt_sb[:, :].rearrange("p (q c) -> p q c", q=CCH),
        wt_d.rearrange("(q p) c -> p q c", p=P),
    )
    if BPC > 1:
        load_T(refT_sbs[1], reft_d, 1, nc.sync)
        load_ref(1, nc.sync)
        load_T(domT_sbs[1], domt_d, 1, nc.gpsimd)
    bias_sb = consts.tile([P, C], F32)
    nc.gpsimd.dma_start(bias_sb[:], bias_d.partition_broadcast(P))
    # rank-1 bias injection operands: ps_z starts at ones^T @ brow = bias
    # broadcast, so the tail groups evict with a scalar copy instead of a
    # vector add (vector is busy with x evictions at the end)
    ones_row = consts.tile([1, P], BF16)
    nc.gpsimd.memset(ones_row[:], 1.0)
    brow_sb = consts.tile([1, C], BF16)
    nc.gpsimd.dma_start(brow_sb[:], biasbf_d[None, :])

    for b in range(BPC):
        domT_sb = domT_sbs[b]
        refT_sb = refT_sbs[b]
        refA_sb = refA_sbs[b]
        refB_sb = refB_sbs[b]

        out_v = out_d[b].rearrange("(n2 two) j -> two n2 j", two=2)

        Pt_sbs = {}
        x_tiles = []

        def mm1_group(h, mi):
            # S^T tile [key(mi) 128, query 512] -> exp into bf16 Pt
            if mi == 0:
                Pt_sbs[h] = p_Pt.tile([P, MCH * 512], BF16, tag="probsT",
                                      name=f"Pt_sb{b}_{h}")
            Pt_sb = Pt_sbs[h]
            ps_s = ps_S.tile([P, 512], F32, tag="ps_s",
                             name=f"ps_s{b}_{h}_{mi}")
            for k in range(CCH):
                nc.tensor.matmul(
                    ps_s[:],
                    refT_sb[:, k * N + mi * P: k * N + (mi + 1) * P],
                    domT_sb[:, k * N + h * 512: k * N + (h + 1) * 512],
                    start=(k == 0), stop=(k == CCH - 1),
                )
            nc.scalar.activation(Pt_sb[:, mi * 512:(mi + 1) * 512],
                                 ps_s[:],
                                 mybir.ActivationFunctionType.Exp,
                                 scale=float(SCALE))

        def mm2_group(h, nl):
            # x tile [query 128, C] = sum_mi Pt(mi)^T @ ref_aug chunk;
            # rowsum rides in column 256 of the A accumulator
            Pt_sb = Pt_sbs[h]
            ps_a = ps_A.tile([P, CA], F32, tag="ps_a",
                             name=f"ps_a{b}_{h}_{nl}")
            ps_b = ps_B.tile([P, CB], F32, tag="ps_b",
                             name=f"ps_b{b}_{h}_{nl}")
            for mi in range(MCH):
                lhsT = Pt_sb[:, mi * 512 + nl * P: mi * 512 + (nl + 1) * P]
                nc.tensor.matmul(ps_a[:], lhsT,
                                 refA_sb[:, mi * CA:(mi + 1) * CA],
                                 start=(mi == 0), stop=(mi == MCH - 1))
                nc.tensor.matmul(ps_b[:], lhsT,
                                 refB_sb[:, mi * CB:(mi + 1) * CB],
                                 start=(mi == 0), stop=(mi == MCH - 1))
            # normalize on eviction: recip of col 256, scale both halves
            recip = p_stats.tile([P, 1], F32, tag="recip",
                                 name=f"recip{b}_{h}_{nl}")
            nc.vector.reciprocal(recip[:], ps_a[:, CB:CA])
            x_t = p_x.tile([P, C], BF16, tag="x", name=f"x_t{b}_{h}_{nl}")
            nc.vector.tensor_scalar_mul(x_t[:, :CB], ps_a[:, :CB], recip[:])
            nc.vector.tensor_scalar_mul(x_t[:, CB:], ps_b[:], recip[:])
            x_tiles.append(x_t)

        def mm3_group(e, cb):
            # out rows (2*cp + e) = x_half_e^T @ wt + bias
            pe_bias = b == BPC - 1 and e == 1 and cb >= JT - 2
            if pe_bias:
                # final group: accumulate in two 256-col PSUM halves (the
                # mm2 A/B banks are free by now) so the first half's store
                # launches while the second half is still accumulating,
                # and the last copy->store chain is half as long
                o_sb = p_out.tile([P, C], F32, tag="out",
                                  name=f"o_sb{b}_{e}_{cb}")
                rows = out_v[e, cb * P:(cb + 1) * P]
                engs = ((nc.gpsimd, nc.sync) if cb % 2 == 0
                        else (nc.sync, nc.gpsimd))
                for half, (pool, eng) in enumerate(
                        ((ps_A, engs[0]), (ps_B, engs[1]))):
                    sl = slice(half * CB, (half + 1) * CB)
                    ps_h = pool.tile([P, CA if half == 0 else CB], F32,
                                     tag="ps_a" if half == 0 else "ps_b",
                                     name=f"ps_zf{half}")
                    nc.tensor.matmul(ps_h[:, :CB], ones_row[:],
                                     brow_sb[:, sl], start=True, stop=False)
                    for q in range(CCH):
                        x_t = x_tiles[e * CCH + q]
                        nc.tensor.matmul(
                            ps_h[:, :CB],
                            x_t[:, cb * P:(cb + 1) * P],
                            wt_sb[:, q * C + half * CB: q * C + half * CB
                                  + CB],
                            start=False, stop=(q == CCH - 1),
                        )
                    nc.scalar.copy(o_sb[:, sl], ps_h[:, :CB])
                    eng.dma_start(rows[:, sl], o_sb[:, sl])
                return
            ps_z = ps_S.tile([P, C], F32, tag="ps_s",
                             name=f"ps_z{b}_{e}_{cb}")
            if pe_bias:
                nc.tensor.matmul(ps_z[:], ones_row[:], brow_sb[:],
                                 start=True, stop=False)
            for q in range(CCH):
                x_t = x_tiles[e * CCH + q]
                nc.tensor.matmul(
                    ps_z[:],
                    x_t[:, cb * P:(cb + 1) * P],
                    wt_sb[:, q * C:(q + 1) * C],
                    start=False if pe_bias else (q == 0),
                    stop=(q == CCH - 1),
                )
            o_sb = p_out.tile([P, C], F32, tag="out",
                              name=f"o_sb{b}_{e}_{cb}")
            if pe_bias:
                nc.scalar.copy(o_sb[:], ps_z[:])
                eng = nc.gpsimd if (e * JT + cb) % 2 == 0 else nc.sync
                eng.dma_start(out_v[e, cb * P:(cb + 1) * P, :], o_sb[:])
            else:
                nc.vector.tensor_add(o_sb[:], ps_z[:], bias_sb[:])
                eng = nc.gpsimd if (e * JT + cb) % 2 == 0 else nc.sync
                eng.dma_start(out_v[e, cb * P:(cb + 1) * P, :], o_sb[:])

        for mi in range(MCH):
            mm1_group(0, mi)
        for mi in range(MCH):
            mm1_group(1, mi)
        for nl in range(4):
            mm2_group(0, nl)
        for cb in range(JT):
            mm3_group(0, cb)
        for nl in range(4):
            mm2_group(1, nl)
        for cb in range(JT):
            mm3_group(1, cb)


_CACHED = {}


def _build():
    key = ("nc", USE_F32R, WARMUP_MMS)
    if key in _CACHED:
        return _CACHED[key]
    nc = bacc.Bacc("TRN2", target_bir_lowering=False, debug=False)
    domt_d = nc.dram_tensor("domt", [BPC, C, N], BF16, kind="ExternalInput").ap()
    reft_d = nc.dram_tensor("reft", [BPC, C, N], BF16, kind="ExternalInput").ap()
    refa_d = nc.dram_tensor("refa", [BPC, N, CB], BF16,
                            kind="ExternalInput").ap()
    refb_d = nc.dram_tensor("refb", [BPC, N, CB], BF16,
                            kind="ExternalInput").ap()
    wt_d = nc.dram_tensor("wt", [C, C], BF16, kind="ExternalInput").ap()
    bias_d = nc.dram_tensor("bias", [C], F32, kind="ExternalInput").ap()
    biasbf_d = nc.dram_tensor("biasbf", [C], BF16, kind="ExternalInput").ap()
    out_d = nc.dram_tensor("out", [BPC, N, C], F32, kind="ExternalOutput").ap()

    with tile.TileContext(nc) as tc:
        _core_kernel(tc, domt_d, reft_d, refa_d, refb_d, wt_d, bias_d,
                     biasbf_d, out_d)
    nc.compile()
    _CACHED[key] = nc
    return nc


LAST_RESULTS = None


def kernel(dom, ref, proj_w, proj_b):
    global LAST_RESULTS
    dom = np.ascontiguousarray(np.asarray(dom, dtype=np.float32))
    ref = np.ascontiguousarray(np.asarray(ref, dtype=np.float32))
    wt = np.ascontiguousarray(
        np.asarray(proj_w, dtype=np.float32).T.astype(ml_dtypes.bfloat16))
    bias = np.ascontiguousarray(np.asarray(proj_b, dtype=np.float32))

    ref_bf = ref.astype(ml_dtypes.bfloat16)
    refa = np.ascontiguousarray(ref_bf[:, :, :CB])
    refb = np.ascontiguousarray(ref_bf[:, :, CB:])
    domt = np.ascontiguousarray(
        dom.transpose(0, 2, 1).astype(ml_dtypes.bfloat16))
    reft = np.ascontiguousarray(
        ref.transpose(0, 2, 1).astype(ml_dtypes.bfloat16))
    nc = _build()
    in_maps = [
        {
            "domt": domt[c * BPC:(c + 1) * BPC],
            "reft": reft[c * BPC:(c + 1) * BPC],
            "refa": refa[c * BPC:(c + 1) * BPC],
            "refb": refb[c * BPC:(c + 1) * BPC],
            "wt": wt,
            "bias": bias,
            "biasbf": bias.astype(ml_dtypes.bfloat16),
        }
        for c in range(CORES)
    ]
    res = run_bass_kernel_spmd(nc, in_maps, list(range(CORES)))
    LAST_RESULTS = res
    if res.exec_time_ns is not None:
        print(f"HW exec time: {res.exec_time_ns} ns")
    return np.concatenate([r["out"] for r in res.results], axis=0)
